# revision 10
# baseline (speedup 1.0000x reference)
"""Trainium2 Bass kernel for nn_DirectedEdgeDecoder (gnn_message_passing) — v2.

  out[e] = W2 . relu(concat(z1[row_e], z2[col_e]) @ W1 + b1) + b2

Rewrite 1 (as baseline): first layer is linear over the concat, so precompute
per-node projections u1 = z1 @ W1[:D] + b1, u2 = z2 @ W1[D:] (8 values/node).

Rewrite 2 (new): avoid per-edge indirect DMAs (this build only supports one
index per partition per instruction, ~512ns each). Instead:
  - shard edges to cores by row-quantiles, so each core's rows live in a
    12.5k-node range with mean degree 8;
  - r-side: split nodes into pseudo-nodes of degree <=4; ONE repeat-AP direct
    DMA per quarter expands u1[r] into a DRAM slot array X (one 256B-strided
    row per edge slot, slots grouped 4-per-pseudo-node);
  - c-side: per slot-quarter, bucket columns by in-quarter degree {1,2,3,6};
    affine repeat-AP DMAs expand u2[c] into SBUF token streams in c-order;
  - bulk dma_scatter_add (int16 idx, 1920 tokens/instruction, round-robin
    SWDGE queues) adds each token into its edge slot row of X;
  - strided readback of X + relu + .W2 + reduce gives per-slot outputs.
All values move on-device; the host only prepares index/bucket metadata and
reorders node tables (layouts are input-independent so kernels compile once).
"""
import numpy as np
import concourse.bass as bass
import concourse.mybir as mybir
import concourse.tile as tile
from concourse import bacc
from concourse.bass_utils import run_bass_kernel_spmd

P = 128
N_CORES = 8
N_NODES = 100000
N_EDGES = 800000
D = 128
H = 8

f32 = mybir.dt.float32
f16 = mybir.dt.float16
i16 = mybir.dt.int16
i32 = mybir.dt.int32

# ------------------------------------------------------------- configuration

class Cfg:
    """Geometry of kernel B. All sizes are compile-time constants."""

    def __init__(self, small=False):
        self.NQ = 4                       # slot quarters per core
        if small:
            self.D1 = 4                   # u1 pseudo-node degree
            self.PN_Q = 192               # u1 pseudo-nodes per quarter
            self.TRASH = 128              # trash rows per quarter
            self.CAP2 = {1: 512, 2: 128, 3: 128, 6: 128}
            self.CHUNK = 256              # scatter tokens per instruction
            self.NCHUNK = 8               # chunks per quarter
        else:
            self.D1 = 4
            self.PN_Q = 7616
            self.TRASH = 256
            self.CAP2 = {1: 20736, 2: 2944, 3: 384, 6: 128}
            self.CHUNK = 1920
            self.NCHUNK = 15
        self.REAL_Q = self.PN_Q * self.D1            # real slots / quarter
        self.SQ = self.REAL_Q + self.TRASH           # rows / quarter in X
        assert self.SQ % 128 == 0 and self.SQ <= 32768
        self.S = self.NQ * self.SQ                   # total X rows
        self.NTOKQ = self.CHUNK * self.NCHUNK        # token slots / quarter
        assert self.NTOKQ % 128 == 0 and self.CHUNK % 128 == 0
        self.TOK2 = {d: c * d for d, c in self.CAP2.items()}
        assert all(v % 128 == 0 for v in self.TOK2.values())
        self.NTOK_USED = sum(self.TOK2.values())
        assert self.NTOK_USED <= self.NTOKQ
        # token k-columns (of 128-token groups) per bucket, in order 1,2,3,6
        self.BUCKETS = [1, 2, 3, 6]
        self.KW = {d: self.TOK2[d] // 128 for d in self.BUCKETS}
        self.KB = {}
        kb = 0
        for d in self.BUCKETS:
            self.KB[d] = kb
            kb += self.KW[d]
        self.KUSED = kb                              # used k-cols / quarter
        self.KTOT = self.NTOKQ // 128
        self.ROWS2_Q = sum(self.CAP2.values())       # U2c rows per quarter
        assert self.TRASH % self.D1 == 0
        # u1 expansion covers trash rows too (zero-pad pseudo-nodes)
        self.ROWS1_Q = self.PN_Q + self.TRASH // self.D1
        self.XCOL = 128                              # f16 per X row (256B)
        self.IDXW = self.NTOKQ // 16                 # idx cols per quarter


CFG = Cfg(small=False)
CFG_SMALL = Cfg(small=True)

# ---------------------------------------------------------------- kernel A
# (same as baseline but emits fp16 u tables)

NC_NODES = N_NODES // N_CORES          # 12500 nodes per core
KN = 98                                # node chunks per core
NP = KN * P                            # 12544 padded nodes per core


def _new_nc(n_queues=1):
    return bacc.Bacc(
        "TRN2", target_bir_lowering=False, debug=False, num_devices=N_CORES,
        num_swdge_queues=n_queues,
    )


def build_precompute():
    """Per-core: u[t] = zT[t].T @ W1[t*128:(t+1)*128] (+ b1 if t == 0), fp16 out.

    Inputs : z1T [128, NP] f32 (z1 shard, transposed, padded), z2T likewise,
             W1 [256, 8] f32, b1 [1, 8] f32
    Output : u [2, NP, 8] f16 -- row r = p*KN + k holds node m = k*128 + p
    """
    nc = _new_nc()
    z1T = nc.declare_dram_parameter("z1T", [P, NP], f32, isOutput=False)
    z2T = nc.declare_dram_parameter("z2T", [P, NP], f32, isOutput=False)
    W1 = nc.declare_dram_parameter("W1", [2 * D, H], f32, isOutput=False)
    b1 = nc.declare_dram_parameter("b1", [1, H], f32, isOutput=False)
    u = nc.declare_dram_parameter("u", [2, NP, H], f16, isOutput=True)

    CH = 7
    CW = CH * P
    NLOAD = KN // CH

    with tile.TileContext(nc) as tc:
        with (
            tc.tile_pool(name="const", bufs=1) as const_pool,
            tc.tile_pool(name="zin", bufs=6) as zin_pool,
            tc.tile_pool(name="acc", bufs=2) as acc_pool,
            tc.tile_pool(name="psum", bufs=4, space="PSUM") as psum_pool,
        ):
            w1sb = const_pool.tile([P, 2 * H], f32)
            for t in range(2):
                nc.sync.dma_start(
                    out=w1sb[:, t * H:(t + 1) * H], in_=W1[t * P:(t + 1) * P, :]
                )
            b1sb = const_pool.tile([P, H], f32)
            nc.sync.dma_start(out=b1sb[:], in_=b1[:].to_broadcast([P, H]))

            for t, zT in enumerate((z1T, z2T)):
                u_acc = acc_pool.tile([P, KN * H], f16, tag="u_acc")
                for j in range(NLOAD):
                    ztile = zin_pool.tile([P, CW], f32, tag="ztile")
                    eng = (nc.sync, nc.scalar, nc.gpsimd)[j % 3]
                    eng.dma_start(out=ztile[:], in_=zT[:, j * CW:(j + 1) * CW])
                    ps = psum_pool.tile([P, CH * H], f32, tag="ps")
                    for i in range(CH):
                        nc.tensor.matmul(
                            out=ps[:, i * H:(i + 1) * H],
                            lhsT=ztile[:, i * P:(i + 1) * P],
                            rhs=w1sb[:, t * H:(t + 1) * H],
                            start=True, stop=True,
                        )
                    if t == 0:
                        nc.vector.tensor_tensor(
                            out=u_acc[:, j * CH * H:(j + 1) * CH * H],
                            in0=ps[:].rearrange("p (c h) -> p c h", h=H),
                            in1=b1sb[:].unsqueeze(1).to_broadcast([P, CH, H]),
                            op=mybir.AluOpType.add,
                        )
                    else:
                        nc.vector.tensor_copy(
                            out=u_acc[:, j * CH * H:(j + 1) * CH * H], in_=ps[:]
                        )
                nc.sync.dma_start(
                    out=u[t].rearrange("(p k) h -> p (k h)", p=P),
                    in_=u_acc[:],
                )
    nc.compile()
    return nc


# ---------------------------------------------------------------- kernel B

def build_scatter_kernel(cfg=CFG):
    """Per-core edge decoder via expansion + scatter-add.

    Inputs : U1c [NQ*ROWS1_Q(+pad), 8] f16  (pseudo-node table, quarter-major)
             U2c [NQ*ROWS2_Q, 8] f16       (per-quarter degree-bucketed table)
             idx [128, NQ*IDXW] i16        (scatter dst, 16-wrapped, replicated)
             W2 [1, 8] f32, b2 [1, 1] f32
    Output : out [128, NQ*SQ/128] f32      (slot s value at [s//(S/128), s%...])
             layout: quarter q, row-in-quarter t -> out[t % 128, q*(SQ//128) + t//128]
    """
    nc = _new_nc(n_queues=1)
    R1 = cfg.NQ * cfg.ROWS1_Q
    R2 = cfg.NQ * cfg.ROWS2_Q
    U1c = nc.declare_dram_parameter("U1c", [R1, H], f16, isOutput=False)
    U2c = nc.declare_dram_parameter("U2c", [R2, H], f16, isOutput=False)
    idx = nc.declare_dram_parameter("idx", [P, cfg.NQ * cfg.IDXW], i16, isOutput=False)
    W2 = nc.declare_dram_parameter("W2", [1, H], f32, isOutput=False)
    b2 = nc.declare_dram_parameter("b2", [1, 1], f32, isOutput=False)
    KOUT = cfg.SQ // 128                 # readback cols per quarter
    out = nc.declare_dram_parameter("out", [P, cfg.NQ * KOUT], f32, isOutput=True)

    X = [
        nc.dram_tensor(f"Xscratch{q}", (cfg.SQ, cfg.XCOL), f16, kind="Internal")
        for q in range(cfg.NQ)
    ]

    with tile.TileContext(nc) as tc:
        with (
            tc.tile_pool(name="const", bufs=1) as const_pool,
            tc.tile_pool(name="tok", bufs=1) as tok_pool,
            tc.tile_pool(name="rb", bufs=3) as rb_pool,
            tc.tile_pool(name="cmp", bufs=3) as cmp_pool,
        ):
            # ---- constants / idx
            idxs = const_pool.tile([P, cfg.NQ * cfg.IDXW], i16)
            for q in range(cfg.NQ):
                nc.gpsimd.dma_start(
                    out=idxs[:, q * cfg.IDXW:(q + 1) * cfg.IDXW],
                    in_=idx[:, q * cfg.IDXW:(q + 1) * cfg.IDXW],
                )
            w2f = const_pool.tile([P, H], f32)
            nc.sync.dma_start(out=w2f[:], in_=W2[:].to_broadcast([P, H]))
            w2sb = const_pool.tile([P, H], f16)
            nc.vector.tensor_copy(out=w2sb[:], in_=w2f[:])
            b2sb = const_pool.tile([P, 1], f32)
            nc.sync.dma_start(out=b2sb[:], in_=b2[:].to_broadcast([P, 1]))

            # ---- u1 expansion into X rows (one DMA per quarter, DRAM->DRAM)
            for q in range(cfg.NQ):
                eng = (nc.scalar, nc.sync)[q % 2]
                eng.dma_start(
                    out=X[q][:, 0:H]
                    .rearrange("(n d) h -> n d h", d=cfg.D1),
                    in_=U1c[q * cfg.ROWS1_Q:(q + 1) * cfg.ROWS1_Q, :]
                    .unsqueeze(1)
                    .to_broadcast([cfg.ROWS1_Q, cfg.D1, H]),
                )

            # ---- u2 expansion into SBUF token streams (per quarter, bucket)
            Y = []
            for q in range(cfg.NQ):
                yq = tok_pool.tile([P, cfg.KTOT * H], f16, tag=f"y{q}")
                Y.append(yq)
                secbase = q * cfg.ROWS2_Q
                for di, d in enumerate(cfg.BUCKETS):
                    npp = cfg.CAP2[d] // 128
                    kb = cfg.KB[d]
                    eng = (nc.sync, nc.scalar)[(q * 4 + di) % 2]
                    eng.dma_start(
                        out=yq[:, kb * H:(kb + cfg.KW[d]) * H]
                        .rearrange("p (n d h) -> p n d h", d=d, h=H),
                        in_=U2c[secbase: secbase + cfg.CAP2[d], :]
                        .rearrange("(p n) h -> p n h", p=P)
                        .unsqueeze(2)
                        .to_broadcast([P, npp, d, H]),
                    )
                    secbase += cfg.CAP2[d]
                if cfg.KUSED < cfg.KTOT:
                    nc.vector.memset(yq[:, cfg.KUSED * H:], 0.0)

            # ---- scatter-add tokens into X
            KCH = cfg.CHUNK // 128       # token k-cols per chunk
            for q in range(cfg.NQ):
                for j in range(cfg.NCHUNK):
                    nc.gpsimd.dma_scatter_add(
                        out_ap=X[q][:, 0:H],
                        in_ap=Y[q][:, j * KCH * H:(j + 1) * KCH * H]
                        .rearrange("p (k h) -> p k h", h=H),
                        idxs_ap=idxs[
                            :, q * cfg.IDXW + j * (cfg.CHUNK // 16):
                            q * cfg.IDXW + (j + 1) * (cfg.CHUNK // 16)
                        ],
                        num_idxs=cfg.CHUNK,
                        num_idxs_reg=cfg.CHUNK,
                        elem_size=H,
                        elem_step=cfg.XCOL,
                        queue_num=0,
                    )

            # ---- readback + relu + .W2 + reduce (+b2), two pieces per quarter
            KH = KOUT // 2
            for q in range(cfg.NQ):
                for hpiece in range(2):
                    rb = rb_pool.tile([P, KH * H], f16, tag="rb")
                    r0 = hpiece * (cfg.SQ // 2)
                    eng = (nc.sync, nc.scalar)[(q * 2 + hpiece) % 2]
                    eng.dma_start(
                        out=rb[:].rearrange("p (k h) -> p k h", h=H),
                        in_=X[q][r0:r0 + cfg.SQ // 2, 0:H]
                        .rearrange("(p k) h -> p k h", p=P),
                    )
                    nc.scalar.activation(
                        out=rb[:], in_=rb[:],
                        func=mybir.ActivationFunctionType.Relu,
                    )
                    veng = nc.gpsimd if q == cfg.NQ - 1 else nc.vector
                    prod = cmp_pool.tile([P, KH * H], f16, tag="prod")
                    veng.tensor_tensor(
                        out=prod[:].rearrange("p (k h) -> p k h", h=H),
                        in0=rb[:].rearrange("p (k h) -> p k h", h=H),
                        in1=w2sb[:].unsqueeze(1).to_broadcast([P, KH, H]),
                        op=mybir.AluOpType.mult,
                    )
                    acc = cmp_pool.tile([P, KH], f32, tag="acc")
                    nc.vector.tensor_reduce(
                        out=acc[:],
                        in_=prod[:].rearrange("p (k h) -> p k h", h=H),
                        axis=mybir.AxisListType.X,
                        op=mybir.AluOpType.add,
                    )
                    veng.tensor_tensor(
                        out=acc[:],
                        in0=acc[:],
                        in1=b2sb[:].to_broadcast([P, KH]),
                        op=mybir.AluOpType.add,
                    )
                    nc.sync.dma_start(
                        out=out[:, q * KOUT + hpiece * KH:
                                q * KOUT + (hpiece + 1) * KH],
                        in_=acc[:],
                    )
    nc.compile()
    return nc


# ------------------------------------------------------------ host planning

def plan_core(rows, cols, cfg=CFG):
    """Plan one core's slot/token layout.

    rows, cols: int arrays [E_c] of u-table row ids (r in core's node range,
    c in 0..N_NODES) for the core's edges, in original edge order.

    Returns dict with:
      u1_rows  [NQ*ROWS1_Q] int  : U1c row -> source node id (-1 = zero pad)
      u2_rows  [NQ*ROWS2_Q] int  : U2c row -> source node id (-1 = zero pad)
      idx      [16, NQ*IDXW] i16 : scatter dst (local row in quarter)
      slot_of  [E_c] int         : global slot id of each edge
    """
    E = len(rows)
    assert len(cols) == E

    # ---- u1 side: group edges by row node, split into deg<=D1 pseudo-nodes
    order = np.argsort(rows, kind="stable")
    srows = rows[order]
    # segment boundaries
    uniq, starts = np.unique(srows, return_index=True)
    ends = np.append(starts[1:], E)

    pseudo_node = []        # source node id per pseudo-node
    pseudo_edges = []       # list of edge-id arrays (<= D1 each)
    for n, s, e in zip(uniq, starts, ends):
        ecnt = e - s
        for off in range(0, ecnt, cfg.D1):
            pseudo_node.append(n)
            pseudo_edges.append(order[s + off: s + off + min(cfg.D1, ecnt - off)])
    NPN = len(pseudo_node)
    assert NPN <= cfg.NQ * cfg.PN_Q, (NPN, cfg.NQ * cfg.PN_Q)

    u1_rows = np.full(cfg.NQ * cfg.ROWS1_Q, -1, dtype=np.int64)
    slot_of = np.full(E, -1, dtype=np.int64)
    qs = [[] for _ in range(cfg.NQ)]     # pseudo-node ids per quarter
    for i in range(NPN):
        qs[min(i // cfg.PN_Q, cfg.NQ - 1)].append(i)
    for q in range(cfg.NQ):
        for slot_j, i in enumerate(qs[q]):
            u1_rows[q * cfg.ROWS1_Q + slot_j] = pseudo_node[i]
            base = q * cfg.SQ + slot_j * cfg.D1
            for t, eid in enumerate(pseudo_edges[i]):
                slot_of[eid] = base + t
    assert (slot_of >= 0).all()

    # ---- u2 side: per quarter, bucket cols by in-quarter degree
    quarter_of = slot_of // cfg.SQ
    u2_rows = np.full(cfg.NQ * cfg.ROWS2_Q, -1, dtype=np.int64)
    idx16 = np.full((16, cfg.NQ * cfg.IDXW), 0, dtype=np.int16)
    # default all tokens -> trash row (last row of the quarter)
    trash_local = cfg.SQ - 1
    for q in range(cfg.NQ):
        idx16[:, q * cfg.IDXW:(q + 1) * cfg.IDXW] = trash_local

    for q in range(cfg.NQ):
        m = quarter_of == q
        qcols = cols[m]
        qeids = np.nonzero(m)[0]
        order_c = np.argsort(qcols, kind="stable")
        sc = qcols[order_c]
        uniq_c, st_c = np.unique(sc, return_index=True)
        en_c = np.append(st_c[1:], len(sc))
        # chunks of <= 6 (largest bucket), then bucket by chunk size
        chunks = {d: [] for d in cfg.BUCKETS}  # (node, edge-id array)
        for n, s, e in zip(uniq_c, st_c, en_c):
            for off in range(s, e, 6):
                grp = order_c[off: min(off + 6, e)]
                csz = len(grp)
                d = next(b for b in cfg.BUCKETS if b >= csz)
                chunks[d].append((n, qeids[grp]))
        rowbase = q * cfg.ROWS2_Q
        for d in cfg.BUCKETS:
            cap = cfg.CAP2[d]
            lst = chunks[d]
            assert len(lst) <= cap, (q, d, len(lst), cap)
            npp = cap // 128
            for slot_j, (n, eids) in enumerate(lst):
                u2_rows[rowbase + slot_j] = n
                # token (p, k) for this pseudo-node: p = slot_j // npp,
                # k = KB[d] + (slot_j % npp) * d + rep
                pp = slot_j // npp
                kk = cfg.KB[d] + (slot_j % npp) * d
                for rep, eid in enumerate(eids):
                    k = kk + rep
                    i_tok = k * 128 + pp          # token index in quarter
                    local = slot_of[eid] - q * cfg.SQ
                    idx16[i_tok % 16, q * cfg.IDXW + i_tok // 16] = local
            rowbase += cap

    return {
        "u1_rows": u1_rows,
        "u2_rows": u2_rows,
        "idx": idx16,
        "slot_of": slot_of,
    }


def shard_edges(edge_row):
    """r-quantile sharding: returns (node_hi[8], edge_core[E]) where core c owns
    nodes in [node_hi[c-1], node_hi[c]) and all edges whose row is in range."""
    counts = np.bincount(edge_row, minlength=N_NODES)
    cum = np.cumsum(counts)
    node_hi = np.zeros(N_CORES, dtype=np.int64)
    tgt = N_EDGES / N_CORES
    for c in range(N_CORES - 1):
        node_hi[c] = np.searchsorted(cum, tgt * (c + 1))
    node_hi[N_CORES - 1] = N_NODES
    edge_core = np.searchsorted(node_hi, edge_row, side="right")
    return node_hi, edge_core


# --------------------------------------------------------------- host glue

def node_to_urow(n):
    """node id -> row in kernel A's per-core u output (c, r) with
    r = (m % P) * KN + m // P, m = n % NC_NODES, c = n // NC_NODES."""
    c = n // NC_NODES
    m = n % NC_NODES
    return c, (m % P) * KN + m // P


def gather_u(uA, table, nodes):
    """uA: list per core of [2, NP, 8] f16 arrays. Returns [len(nodes), 8] f16,
    zeros where nodes == -1."""
    res = np.zeros((len(nodes), H), dtype=np.float16)
    valid = nodes >= 0
    n = nodes[valid]
    c, r = node_to_urow(n)
    vals = np.empty((len(n), H), dtype=np.float16)
    for cc in range(N_CORES):
        m = c == cc
        if m.any():
            vals[m] = uA[cc][table][r[m]]
    res[valid] = vals
    return res


def prep_precompute_inputs(z1, z2, W1, b1):
    W1 = np.ascontiguousarray(W1, dtype=np.float32)
    b1 = np.ascontiguousarray(b1, dtype=np.float32).reshape(1, H)
    in_maps = []
    for c in range(N_CORES):
        m = {}
        for name, z in (("z1T", z1), ("z2T", z2)):
            sh = np.zeros((NP, D), dtype=np.float32)
            sh[:NC_NODES] = z[c * NC_NODES:(c + 1) * NC_NODES]
            m[name] = np.ascontiguousarray(sh.T)
        m["W1"] = W1
        m["b1"] = b1
        in_maps.append(m)
    return in_maps


_CACHE = {}


def _get_kernels():
    if "a" not in _CACHE:
        _CACHE["a"] = build_precompute()
        _CACHE["b"] = build_scatter_kernel()
    return _CACHE["a"], _CACHE["b"]


LAST_IN_MAPS = {}


def kernel(z1, z2, edge_index, W1, b1, W2, b2):
    z1 = np.asarray(z1, dtype=np.float32)
    z2 = np.asarray(z2, dtype=np.float32)
    edge_index = np.asarray(edge_index)
    W2 = np.ascontiguousarray(np.asarray(W2, dtype=np.float32).reshape(1, H))
    b2 = np.ascontiguousarray(np.asarray(b2, dtype=np.float32)).reshape(1, 1)
    cfg = CFG

    nc_a, nc_b = _get_kernels()
    core_ids = list(range(N_CORES))

    # ---- launch A
    in_maps_a = prep_precompute_inputs(z1, z2, W1, b1)
    LAST_IN_MAPS["a"] = in_maps_a
    res_a = run_bass_kernel_spmd(nc_a, in_maps_a, core_ids)
    uA = [res_a.results[c]["u"] for c in range(N_CORES)]

    # ---- plan + launch B
    row = np.asarray(edge_index[0], dtype=np.int64)
    col = np.asarray(edge_index[1], dtype=np.int64)
    node_hi, edge_core = shard_edges(row)

    in_maps, plans, eids = [], [], []
    for c in range(N_CORES):
        m = edge_core == c
        eid = np.nonzero(m)[0]
        plan = plan_core(row[eid], col[eid], cfg)
        U1c = gather_u(uA, 0, plan["u1_rows"])
        U2c = gather_u(uA, 1, plan["u2_rows"])
        in_maps.append({
            "U1c": np.ascontiguousarray(U1c),
            "U2c": np.ascontiguousarray(U2c),
            "idx": np.ascontiguousarray(np.tile(plan["idx"], (8, 1))),
            "W2": W2,
            "b2": b2,
        })
        plans.append(plan)
        eids.append(eid)
    LAST_IN_MAPS["b"] = in_maps
    res_b = run_bass_kernel_spmd(nc_b, in_maps, core_ids)

    # ---- unpermute
    out = np.empty((N_EDGES, 1), dtype=np.float32)
    for c in range(N_CORES):
        o = res_b.results[c]["out"]          # [128, NQ*KOUT]
        vals = slot_values(o, plans[c]["slot_of"], cfg)
        out[eids[c], 0] = vals
    return out


def slot_values(o, slot, cfg=CFG):
    """Map kernel B 'out' [128, NQ*KOUT] to per-slot values."""
    KOUT = cfg.SQ // 128
    KH = KOUT // 2
    q, t = slot // cfg.SQ, slot % cfg.SQ
    hp, t2 = t // (cfg.SQ // 2), t % (cfg.SQ // 2)
    return o[t2 // KH, q * KOUT + hp * KH + t2 % KH]
